# revision 30
# baseline (speedup 1.0000x reference)
"""BoxFilter (9x9 box-sum, clamped borders) Trainium2 Bass kernel.

Input  x: [16, 3, 1024, 1024] f32, r=4 (hardcoded).
Output y: same shape; y[b,c,i,j] = sum of x[b,c,u,v] over the
(2r+1)x(2r+1) window centered at (i,j), clipped to the image bounds
(exactly what the reference's cumsum+diff computes).

Sharding: pure data parallel over 8 cores, 6 of the 48 images each.

The 2e-2 rel-err gate leaves huge headroom, so everything runs in
fp16 (input quantization + fp16 output give ~6e-4 rel err): input
DMA is 2 B/elem (no hi/lo split) and the output DMA is fp16 too,
upcast to f32 on the host.

Per-core pipeline (per image, 9 overlapping 128-row slabs). Each slab
is processed by one of three sub-pipelines, mixed to balance the
Vector, Tensor and Scalar engines (measured per-slab costs in ns:
scan ~2280 V / extraction ~1110 S / matmul ~215 per 512-col stream T,
K-independent, so the H-band contraction is free):

  A  (V-heavy): H-band matmul (2 MM) -> PSUM f32; ScalarE extracts to
     a zero-padded fp16 tile; one merged tensor_tensor_scan computes
     the 9-window running box along W (state=(y[t]+state)-y[t-9] over
     1028 steps; leading/trailing zero pads make both clamps
     automatic).
  B  (T-heavy): the full 2D box via 9 accumulating band-matmuls over
     column-shifted views of the zero-padded input slab (each W shift
     costs one 512-col stream).  ScalarE extraction is the final
     output.  No Vector work.
  B2 (T+S): box9 = box3 o box3: 3 shifted band-matmuls -> t3[m] =
     T[m-1]+T[m]+T[m+1] for m in [-3, 1020] (exactly 2 PSUM banks),
     extract to fp16, then out[j] = t3[j-3]+t3[j]+t3[j+3] via 3
     shifted identity-matmuls (j < 1018); the last 6 output cols are
     patched with 9 direct N=6 band-matmuls so everything fits 2-bank
     PSUM tiles.  Level 2 is emitted TWO SLABS LATE so its dependency
     on the level-1 extraction never head-of-line blocks the strict
     FIFO PE queue.

Input DMAs issue from the Sync queue (HWDGE, alternating with GpSimd
for the first image to shorten the ramp); output DMAs issue from the
GpSimd queue (SWDGE), alternating with Sync for the last image to
halve the end-of-kernel trigger drain.
"""

import os
import numpy as np

from concourse import bass, mybir, tile, bacc
from concourse.bass_utils import run_bass_kernel_spmd

F32 = mybir.dt.float32
FP16 = mybir.dt.float16
H, W = 1024, 1024
N_CORES = 8
IPC = 6  # images per core: (16*3)/8
R = 4
D = 2 * R + 1  # 9
XCOLS = W + 2 * R  # 1032: input slab with R zero cols each side

# slabs: (row0, nrows, out0, nouts, band_col)
_SLABS = (
    [(0, 128, 0, 124, 0)]
    + [(120 * i, 128, 120 * i + 4, 120, 124) for i in range(1, 8)]
    + [(960, 64, 964, 60, 244)]
)
_BAND_COLS = 304  # 124 + 120 + 60

# slab type per (img, slab): A=scan, B=9-shift matmul, B2=3+3 two-level
_PA = ["A", "B2", "A", "B2", "A", "B", "A", "B2", "A"]  # A5 B1 B2x3
_PA2 = ["A", "B2", "A", "A", "A", "B", "A", "B2", "A"]  # A6 B1 B2x2
_PB = ["A", "B2", "A", "A", "B", "A", "B2", "A", "A"]   # A6 B1 B2x2
_PC = ["A", "B2", "A", "A", "A", "B2", "A", "A", "A"]   # A7 B0 B2x2
_PATTERNS = [_PC, _PB, _PA2, _PB, _PA2, _PA]  # A36 B5 B2x13; V-heavy first, V-light last


def _band_matrix() -> np.ndarray:
    bands = np.zeros((128, _BAND_COLS), np.float16)
    for row0, nrows, out0, nouts, bc in (_SLABS[0], _SLABS[1], _SLABS[8]):
        for j in range(nouts):
            h_out = out0 + j
            lo = max(0, h_out - R) - row0
            hi = min(H - 1, h_out + R) - row0
            bands[lo : hi + 1, bc + j] = 1.0
    return bands


_CACHE: dict = {}

# Set by the most recent kernel() call (for test harnesses).
LAST_RESULTS = None


def _build():
    nc = bacc.Bacc(
        "TRN2", target_bir_lowering=False, debug=False, enable_asserts=False
    )
    x_d = nc.dram_tensor("x", [IPC, H, W], FP16, kind="ExternalInput").ap()
    bands_d = nc.dram_tensor(
        "bands", [128, _BAND_COLS], FP16, kind="ExternalInput"
    ).ap()
    ident_d = nc.dram_tensor("ident", [128, 128], FP16, kind="ExternalInput").ap()
    y_d = nc.dram_tensor("y", [IPC, H, W], FP16, kind="ExternalOutput").ap()

    ADD = mybir.AluOpType.add
    SUB = mybir.AluOpType.subtract

    XPAD_BUFS = 12
    YT_BUFS = 8
    BX_BUFS = 6
    OUT_BUFS = 5
    T3_BUFS = 4

    with tile.TileContext(nc) as tc:
        with (
            tc.tile_pool(name="const", bufs=1) as const_pool,
            tc.tile_pool(name="xin", bufs=XPAD_BUFS) as in_pool,
            tc.tile_pool(name="ps2", bufs=4, space="PSUM") as ps2_pool,
            tc.tile_pool(name="yrow", bufs=YT_BUFS) as y_pool,
            tc.tile_pool(name="box", bufs=BX_BUFS) as box_pool,
            tc.tile_pool(name="t3", bufs=T3_BUFS) as t3_pool,
            tc.tile_pool(name="outb", bufs=OUT_BUFS) as out_pool,
        ):
            # constants go via the Scalar queue (HWDGE on TRN2, idle at
            # start) so the first input slab is the Sync queue's first
            # trigger -- shortens the LDWEIGHTS->matmul->scan start chain
            bands_t = const_pool.tile([128, _BAND_COLS], FP16)
            nc.scalar.dma_start(bands_t[:], bands_d[:])
            ident_t = const_pool.tile([128, 128], FP16)
            nc.scalar.dma_start(ident_t[:], ident_d[:])

            def out_queue(idx):
                if idx >= 45:
                    return nc.sync
                if idx >= 36:
                    return nc.gpsimd if idx % 2 == 0 else nc.sync
                return nc.gpsimd

            def stage2_b2(st):
                """B2 level 2: out[j] = t3[j-3] + t3[j] + t3[j+3]."""
                img, out0, nouts, nrows, band_ap, xpad, t3b, idx = st
                ps = ps2_pool.tile([128, 1024], F32, tag="ps2")
                # Ident groups: cols [0,512) (bank0) and [512,1018) (bank1);
                # tail cols [1018,1024) (bank1) from 9 direct N=6 band-MMs
                # interleaved so each tiny MM's LDWEIGHTS hides behind a
                # 512-col stream.  start=True clears has_written for the
                # WHOLE bank, so the tail group opens bank1 (start=True) and
                # the [512,1018) ident group then uses start=False (its bits
                # are already clear -> first write still overwrites).
                ident_mms = [
                    (0, 512, 0, True, False), (0, 512, 3, False, False),
                    (0, 512, 6, False, True),
                    (512, 506, 0, False, False), (512, 506, 3, False, False),
                    (512, 506, 6, False, True),
                ]
                tail_mms = [(1018 + 0, si) for si in range(D)]
                order = []
                for k in range(6):
                    order.append(("i", ident_mms[k]))
                    order.append(("t", k))
                order += [("t", k) for k in range(6, D)]
                for kind, it in order:
                    if kind == "i":
                        c0, n, s, st, sp = it
                        nc.tensor.matmul(
                            ps[:nouts, c0 : c0 + n],
                            lhsT=ident_t[:nouts, :nouts],
                            rhs=t3b[:nouts, c0 + s : c0 + s + n],
                            start=st,
                            stop=sp,
                            skip_group_check=True,
                        )
                    else:
                        si = it
                        nc.tensor.matmul(
                            ps[:nouts, 1018:1024],
                            lhsT=band_ap,
                            rhs=xpad[:nrows, 1018 + si : 1024 + si],
                            start=(si == 0),
                            stop=(si == D - 1),
                            skip_group_check=True,
                        )
                ob = out_pool.tile([128, W], FP16, tag="outb")
                nc.scalar.copy(ob[:nouts, :], ps[:nouts, :])
                out_queue(idx).dma_start(
                    y_d[img, out0 : out0 + nouts, :], ob[:nouts, :]
                )

            def stage2_b(st):
                """B second half: bank1's 9 shifted MMs + extraction."""
                img, out0, nouts, nrows, band_ap, xpad, ps, idx = st
                for s_ in range(D):
                    nc.tensor.matmul(
                        ps[:nouts, 512:1024],
                        lhsT=band_ap,
                        rhs=xpad[:nrows, s_ + 512 : s_ + 1024],
                        start=(s_ == 0),
                        stop=(s_ == D - 1),
                    )
                ob = out_pool.tile([128, W], FP16, tag="outb")
                nc.scalar.copy(ob[:nouts, :], ps[:nouts, :])
                out_queue(idx).dma_start(
                    y_d[img, out0 : out0 + nouts, :], ob[:nouts, :]
                )

            pending = []
            slab_idx = 0
            for img in range(IPC):
                pat = _PATTERNS[img]
                for si, (row0, nrows, out0, nouts, bc) in enumerate(_SLABS):
                    typ = pat[si]
                    band_ap = bands_t[:nrows, bc : bc + nouts]

                    # input slab with R zero cols each side (pads zeroed
                    # once per physical pool slot)
                    xpad = in_pool.tile([128, XCOLS], FP16, tag="xpad")
                    if slab_idx < XPAD_BUFS:
                        nc.gpsimd.memset(xpad[:, 0:R], 0.0)
                        nc.gpsimd.memset(xpad[:, R + W : XCOLS], 0.0)
                    iq = nc.gpsimd if (slab_idx < 9 and slab_idx % 2 == 1) else nc.sync
                    iq.dma_start(
                        xpad[:nrows, R : R + W],
                        x_d[img, row0 : row0 + nrows, :],
                    )

                    if typ == "A":
                        ps = ps2_pool.tile([128, 1024], F32, tag="ps2")
                        for b in range(2):
                            nc.tensor.matmul(
                                ps[:nouts, b * 512 : (b + 1) * 512],
                                lhsT=band_ap,
                                rhs=xpad[:nrows, R + b * 512 : R + (b + 1) * 512],
                                start=True,
                                stop=True,
                            )
                        # yt: [0:9) zeros, [9:1033) = H-filtered rows,
                        # [1033:1037) zeros (right-border steps of the scan)
                        yt = y_pool.tile([128, W + D + R], FP16, tag="yrow")
                        if slab_idx < YT_BUFS * 2:
                            nc.gpsimd.memset(yt[:, 0:D], 0.0)
                            nc.gpsimd.memset(yt[:, D + W : D + W + R], 0.0)
                        nc.scalar.copy(yt[:nouts, D : D + W], ps[:nouts, :])
                        # merged scan: state = (y[t] + state) - y[t-9]; for
                        # the last 4 steps data0 reads the trailing zeros and
                        # data1 walks the right clamp down from box_end[W-1].
                        bx = box_pool.tile([128, W + R], FP16, tag="box")
                        nc.vector.tensor_tensor_scan(
                            bx[:nouts, 0 : W + R],
                            yt[:nouts, D : D + W + R],
                            yt[:nouts, 0 : W + R],
                            0.0,
                            op0=ADD,
                            op1=SUB,
                        )
                        out_queue(slab_idx).dma_start(
                            y_d[img, out0 : out0 + nouts, :],
                            bx[:nouts, R : R + W],
                        )

                    elif typ == "B":
                        ps = ps2_pool.tile([128, 1024], F32, tag="ps2")
                        for s in range(D):
                            nc.tensor.matmul(
                                ps[:nouts, 0:512],
                                lhsT=band_ap,
                                rhs=xpad[:nrows, s : s + 512],
                                start=(s == 0),
                                stop=(s == D - 1),
                            )
                        pending.append(
                            (slab_idx + 1, stage2_b,
                             (img, out0, nouts, nrows, band_ap, xpad, ps,
                              slab_idx))
                        )

                    else:  # B2 level 1: t3[m], m in [-3, 1020], psum col m+3
                        ps3 = ps2_pool.tile([128, 1024], F32, tag="ps2")
                        for c0 in (0, 512):
                            for s in range(3):
                                nc.tensor.matmul(
                                    ps3[:nouts, c0 : c0 + 512],
                                    lhsT=band_ap,
                                    rhs=xpad[:nrows, s + c0 : s + c0 + 512],
                                    start=(s == 0),
                                    stop=(s == 2),
                                )
                        t3b = t3_pool.tile([128, 1024], FP16, tag="t3b")
                        nc.scalar.copy(t3b[:nouts, :], ps3[:nouts, :])
                        pending.append(
                            (slab_idx + 2, stage2_b2,
                             (img, out0, nouts, nrows, band_ap, xpad, t3b, slab_idx))
                        )

                    while pending and pending[0][0] <= slab_idx:
                        _, fn, st = pending.pop(0)
                        fn(st)
                    slab_idx += 1

            for _, fn, st in pending:
                fn(st)

    nc.compile()
    return nc


def kernel(x: np.ndarray, r) -> np.ndarray:
    global LAST_RESULTS
    x = np.asarray(x, dtype=np.float32)
    assert x.shape == (16, 3, H, W), x.shape
    assert int(r) == R, r

    nc = _CACHE.get("nc")
    if nc is None:
        nc = _CACHE["nc"] = _build()

    xr = x.reshape(N_CORES, IPC, H, W).astype(np.float16)
    bands = _band_matrix()
    ident = np.eye(128, dtype=np.float16)
    in_maps = [
        {"x": np.ascontiguousarray(xr[c]), "bands": bands, "ident": ident}
        for c in range(N_CORES)
    ]

    trace = bool(int(os.environ.get("BOX_TRACE", "0")))
    tmpdir = os.environ.get("BOX_TRACE_DIR") or None
    if tmpdir:
        os.makedirs(tmpdir, exist_ok=True)
    res = run_bass_kernel_spmd(
        nc, in_maps, list(range(N_CORES)), trace=trace, tmpdir=tmpdir
    )
    LAST_RESULTS = res
    y = np.stack([res.results[c]["y"] for c in range(N_CORES)])
    return y.reshape(16, 3, H, W).astype(np.float32)
